# revision 2
# baseline (speedup 1.0000x reference)
"""Trainium2 Bass kernel for nn_ConditionalFeedForward (MoE top-2 routing).

Strategy: expert-parallel across the 8 NeuronCores — core e owns expert e's
weights. Host (numpy) gathers each expert's routed tokens (multi-hot
routing_map), pads to a common capacity CAP, and pre-transposes/pre-tiles
operands into PE-friendly layouts. Each core computes, for its expert:

    hT = silu(w1 @ xT) * (w3 @ xT)          # [FFN, CAP] staged via DRAM
    yT = w2 @ hT                            # [DIM, CAP]

with float32r matmuls (full PE rate, ~1e-4 relative error), SwiGLU fused on
ScalarE (Silu) + VectorE (mul). Host scatter-adds gate-weighted outputs back
to the full [N_TOKENS, DIM] result.
"""

import os
import numpy as np

import concourse.bacc as bacc
import concourse.mybir as mybir
import concourse.tile as tile
from concourse.bass_utils import run_bass_kernel_spmd

# Problem constants (hardcoded per harness contract)
NUM_EXPERTS = 8
DIM = 2048
FFN = 5632
N_CORES = 8
KD = DIM // 128    # 16 contraction tiles for GEMM1/3, output tiles for GEMM2
KF = FFN // 128    # 44 ffn chunks

F32 = mybir.dt.float32
F32R = mybir.dt.float32r

# Compiled program cache keyed by CAP
_PROGRAMS = {}

# Filled by the last kernel() call when BASS_KERNEL_TRACE=1 (for test.py)
LAST_EXEC_NS = None


def _p1_tiles(cap):
    """Phase-1 token tiles: 512s plus an optional 256 tail (cap % 256 == 0)."""
    tiles = []
    t0 = 0
    while t0 < cap:
        tn = min(512, cap - t0)
        tiles.append((t0, tn))
        t0 += tn
    return tiles


def _p2_blocks(cap):
    """Phase-2 token blocks of <=768 (SBUF-resident hT slab per block)."""
    blocks = []
    b0 = 0
    while b0 < cap:
        bn = min(768, cap - b0)
        blocks.append((b0, bn))
        b0 += bn
    return blocks


def _p2_subtiles(bn):
    """Split a block into PSUM-bank-sized matmul N-tiles (each in [256,512])."""
    if bn <= 512:
        return [(0, bn)]
    assert bn == 768
    return [(0, 512), (512, 256)]


def _build_program(cap):
    nc = bacc.Bacc("TRN2", target_bir_lowering=False, debug=False,
                   num_devices=N_CORES)

    xt_d = nc.dram_tensor("xt", [KD, 128, cap], F32R, kind="ExternalInput")
    w1l_d = nc.dram_tensor("w1l", [KF, 128, KD, 128], F32R, kind="ExternalInput")
    w3l_d = nc.dram_tensor("w3l", [KF, 128, KD, 128], F32R, kind="ExternalInput")
    w2l_d = nc.dram_tensor("w2l", [KD, 128, KF, 128], F32R, kind="ExternalInput")
    yt_d = nc.dram_tensor("yt", [KD, 128, cap], F32, kind="ExternalOutput")
    htb_d = nc.dram_tensor("htb", [KF, 128, cap], F32R, kind="Internal")

    silu = mybir.ActivationFunctionType.Silu
    p1t = _p1_tiles(cap)

    with tile.TileContext(nc) as tc:
        # ---- Phase 1: hT = silu(w1 @ xT) * (w3 @ xT), staged to DRAM ----
        with (
            tc.tile_pool(name="xt", bufs=1) as xpool,
            tc.tile_pool(name="w13", bufs=2) as wpool,
            tc.tile_pool(name="hst", bufs=3) as spool,
            tc.tile_pool(name="ps1", bufs=2, space="PSUM") as psum1,
        ):
            xt_s = xpool.tile([128, KD, cap], F32R)
            for k in range(KD):
                nc.sync.dma_start(xt_s[:, k, :], xt_d[k])
            for f in range(KF):
                w1c = wpool.tile([128, KD, 128], F32R, tag="w1c")
                nc.sync.dma_start(w1c[:], w1l_d[f])
                w3c = wpool.tile([128, KD, 128], F32R, tag="w3c")
                nc.sync.dma_start(w3c[:], w3l_d[f])
                for (t0, tn) in p1t:
                    h1p = psum1.tile([128, tn], F32, tag="h1p")
                    h3p = psum1.tile([128, tn], F32, tag="h3p")
                    for k in range(KD):
                        nc.tensor.matmul(
                            h1p[:], w1c[:, k, :], xt_s[:, k, t0:t0 + tn],
                            start=(k == 0), stop=(k == KD - 1))
                    for k in range(KD):
                        nc.tensor.matmul(
                            h3p[:], w3c[:, k, :], xt_s[:, k, t0:t0 + tn],
                            start=(k == 0), stop=(k == KD - 1))
                    s1 = spool.tile([128, tn], F32, tag="s1")
                    nc.scalar.activation(s1[:], h1p[:], silu)
                    ht = spool.tile([128, tn], F32, tag="ht")
                    nc.vector.tensor_mul(ht[:], s1[:], h3p[:])
                    nc.scalar.dma_start(htb_d[f][:, t0:t0 + tn],
                                        ht[:].bitcast(F32R))

        # ---- Phase 2: yT = w2 @ hT ----
        with (
            tc.tile_pool(name="htk", bufs=1) as hpool,
            tc.tile_pool(name="w2", bufs=2) as w2pool,
            tc.tile_pool(name="yo", bufs=3) as ypool,
            tc.tile_pool(name="ps2", bufs=4, space="PSUM") as psum2,
        ):
            for (b0, bn) in _p2_blocks(cap):
                htk = hpool.tile([128, KF, bn], F32R, tag="htk")
                for k2 in range(KF):
                    nc.sync.dma_start(htk[:, k2, :], htb_d[k2][:, b0:b0 + bn])
                for m in range(KD):
                    w2c = w2pool.tile([128, KF, 128], F32R, tag="w2c")
                    nc.sync.dma_start(w2c[:], w2l_d[m])
                    for (s0, sn) in _p2_subtiles(bn):
                        yp = psum2.tile([128, sn], F32, tag="yp")
                        for k2 in range(KF):
                            nc.tensor.matmul(
                                yp[:], w2c[:, k2, :], htk[:, k2, s0:s0 + sn],
                                start=(k2 == 0), stop=(k2 == KF - 1))
                        yo = ypool.tile([128, sn], F32, tag="yo")
                        nc.vector.tensor_copy(yo[:], yp[:])
                        nc.scalar.dma_start(
                            yt_d[m][:, b0 + s0:b0 + s0 + sn], yo[:])

    nc.compile()
    return nc


def kernel(x, expert_indices, expert_weights, w1, w2, w3):
    global LAST_EXEC_NS
    x = np.ascontiguousarray(np.asarray(x, dtype=np.float32))
    routing = np.asarray(expert_indices)
    probs = np.asarray(expert_weights, dtype=np.float32)
    w1 = np.asarray(w1, dtype=np.float32)
    w2 = np.asarray(w2, dtype=np.float32)
    w3 = np.asarray(w3, dtype=np.float32)
    n_tokens = x.shape[0]

    idxs = [np.flatnonzero(routing[:, e]) for e in range(NUM_EXPERTS)]
    max_count = max(len(i) for i in idxs)
    cap = max(256, -(-max_count // 256) * 256)  # round up to multiple of 256
    assert cap <= 2304, f"unexpectedly imbalanced routing: max_count={max_count}"

    if cap not in _PROGRAMS:
        _PROGRAMS[cap] = _build_program(cap)
    nc = _PROGRAMS[cap]

    in_maps = []
    for e in range(NUM_EXPERTS):
        idx = idxs[e]
        xt = np.zeros((DIM, cap), dtype=np.float32)
        xt[:, :len(idx)] = x[idx].T
        in_maps.append({
            "xt": xt.reshape(KD, 128, cap),
            # W1L[f,p,k,m] = w1[e][f*128+m, k*128+p]
            "w1l": np.ascontiguousarray(
                w1[e].reshape(KF, 128, KD, 128).transpose(0, 3, 2, 1)),
            "w3l": np.ascontiguousarray(
                w3[e].reshape(KF, 128, KD, 128).transpose(0, 3, 2, 1)),
            # W2L[m,p,k2,d] = w2[e][m*128+d, k2*128+p]
            "w2l": np.ascontiguousarray(
                w2[e].reshape(KD, 128, KF, 128).transpose(0, 3, 2, 1)),
        })

    trace = os.environ.get("BASS_KERNEL_TRACE", "0") == "1"
    if trace:
        import importlib.util
        if importlib.util.find_spec("antenv") is None or importlib.util.find_spec(
                "antenv.axon_hooks") is None:
            trace = False  # NTFF hook unavailable in this environment
    res = run_bass_kernel_spmd(
        nc, in_maps, core_ids=list(range(N_CORES)),
        trace=trace, trace_cores=list(range(N_CORES)) if trace else None,
    )
    LAST_EXEC_NS = res.exec_time_ns

    out = np.zeros((n_tokens, DIM), dtype=np.float32)
    for e in range(NUM_EXPERTS):
        idx = idxs[e]
        y_t = res.results[e]["yt"].reshape(DIM, cap)[:, :len(idx)]
        out[idx] += probs[idx, e][:, None] * y_t.T
    return out
